# revision 15
# baseline (speedup 1.0000x reference)
"""Batched dot-product attention (B=32, Lq=Lk=2048, d=dv=64, fp32) on 8 TRN2
NeuronCores.

Data parallel over batch (4 per core). Transposed-layout attention, bf16
matmul operands / f32 PSUM accumulation:

  - q, k, v are cast to bf16 on the Pool engine (GPSIMD cannot touch PSUM on
    real silicon, so it gets all SBUF-side prep).
  - q^T / k^T are built by the DMA xbar transpose engine (dma_start_transpose,
    2-byte dtypes) straight from the natural [p, chunk, d] bf16 staging into
    the pair layout out[j, c, p] = in[p, c*128 + j] (J=128 is the only form
    whose HW semantics match the AP). The PE runs matmuls only (plus a few
    f32 transposes to jump-start batch 0 while the DMA path warms up).
  - S^T chunk [128 (Lk), 256 (Lq)] matmuls use full-128-contraction operands
    with one half zeroed (kte = even k-chunks on rows 0:64 + zeros, kto =
    odd chunks on rows 64:128 + zeros; qa / partition-swapped qb_t supply the
    matching q parity). Real HW rejects 64-row bf16 matmuls, and zero-padding
    is free: matmul cost is out-columns only.
  - exp (16.8M elems/core, the scalar-class bottleneck) is split per
    chunk-pair, ENG_MAP = Act x5 / DVE x3 per q tile:
      Act: true exp activation, bf16 out, scale=1/sqrt(d) folded
      DVE: Schraudolph bit trick, one op: int16(x*A16 + B16) bitcast to bf16
           (mult err < ~3.3%; softmax normalization cancels most of it)
  - out^T [65, 256] accumulates over the 16 Lk chunks in f32 PSUM with
    lhsT = [v_chunk | ones]; row 64 is the softmax denominator for free.
    Final divide + transpose back to [L, 64] happen on host (trivial numpy).
  - The S^T producer runs 6 pairs ahead of the exp/PV consumer (6 one-bank
    S^T tiles + 2 out^T banks = all 8 PSUM banks) so the PE (~88% busy, the
    bottleneck engine) rarely waits; out^T evacuation (Act) is delayed one
    dispatch group so it never blocks the next unit's exp.

k and v use a row permutation (DRAM rearrange "(p c) d") for 2-4KB DMA runs;
the permutation is consistent between k-chunks and v-chunks so the softmax
math is unchanged. q keeps natural order (its order defines output rows).
No max-subtraction: raw logits are within +-70, safely inside both the f32
exp range and the bit-trick's int16 domain.
"""

import sys

if "/opt/trn_rl_repo" not in sys.path:
    sys.path.insert(0, "/opt/trn_rl_repo")

from contextlib import ExitStack

import numpy as np

import concourse.tile as tile
from concourse import bacc, mybir
from concourse.masks import make_identity

B_TOTAL = 32
N_CORES = 8
B = B_TOTAL // N_CORES
L = 2048
D = 64
P = 128
NBLK = L // P  # 16 Lk chunks
LQT = 256
NQT = L // LQT  # 8 q tiles per batch
PAIRS = NBLK // 2  # 8 chunk pairs per q tile
SCALE = 1.0 / float(np.sqrt(D))

F32 = mybir.dt.float32
BF16 = mybir.dt.bfloat16
I16 = mybir.dt.int16
EXP = mybir.ActivationFunctionType.Exp
MULT = mybir.AluOpType.mult
ADD = mybir.AluOpType.add

# Schraudolph bf16 bit trick: int16 bits = trunc(x*A16 + B16); bits viewed as
# bf16 ~= exp(x/8) within ~3.3%.
A16 = float(SCALE * np.log2(np.e) * (2 ** 7))
B16 = float((127.0 - 0.044) * (2 ** 7))

ENG_MAP = "ADAADAAD"  # pair -> engine: Act x5, DVE x3 per q tile
USE_DMAT = True  # DMA xbar transposes (False: PE transposes, for bisecting)
GROUPS = 8  # dispatch groups (1 chunk-pair each) per q tile
ST_AHEAD = 6  # S^T producer lookahead in groups == ps_st bufs


def build_attention_kernel():
    nc = bacc.Bacc("TRN2", target_bir_lowering=False, debug=False)
    q_d = nc.dram_tensor("q", [B, L, D], F32, kind="ExternalInput")
    k_d = nc.dram_tensor("k", [B, L, D], F32, kind="ExternalInput")
    v_d = nc.dram_tensor("v", [B, L, D], F32, kind="ExternalInput")
    o_d = nc.dram_tensor("outt", [B, D + 1, L], F32, kind="ExternalOutput")

    q_r = [q_d.ap()[b].rearrange("(c p) d -> p c d", p=P) for b in range(B)]
    k_r = [k_d.ap()[b].rearrange("(p c) d -> p c d", p=P) for b in range(B)]
    v_r = [v_d.ap()[b].rearrange("(p c) d -> p c d", p=P) for b in range(B)]

    with tile.TileContext(nc) as tc, ExitStack() as ctx:
        const = ctx.enter_context(tc.tile_pool(name="const", bufs=1))
        nat = ctx.enter_context(tc.tile_pool(name="nat", bufs=6))
        bfp = ctx.enter_context(tc.tile_pool(name="bfp", bufs=4))
        ktp = ctx.enter_context(tc.tile_pool(name="ktp", bufs=6))
        qtp = ctx.enter_context(tc.tile_pool(name="qtp", bufs=4))
        vp = ctx.enter_context(tc.tile_pool(name="vp", bufs=2))
        pp = ctx.enter_context(tc.tile_pool(name="pp", bufs=8))
        otp = ctx.enter_context(tc.tile_pool(name="otp", bufs=3))
        ps_st = ctx.enter_context(tc.tile_pool(name="ps_st", bufs=ST_AHEAD, space="PSUM"))
        ps_ot = ctx.enter_context(tc.tile_pool(name="ps_ot", bufs=2, space="PSUM"))

        ident = const.tile([P, P], BF16)
        make_identity(nc, ident[:])
        ones_col = const.tile([P, NBLK, 1], BF16)
        nc.vector.memset(ones_col[:], 1.0)

        # Warm the PE p-state during initial DMA/staging.
        warm = ps_st.tile([P, 2, P], BF16, tag="st", name="warm")
        for w in range(8):
            nc.tensor.transpose(warm[:, w % 2, :], ident[:], ident[:])

        q_bf = [None] * B
        k_bf = [None] * B
        kt_all = [None] * B
        qt_all = [None] * B
        v_sb = [None] * B
        nat_tiles = [None] * B

        def emit_loads(b):
            qn = nat.tile([P, NBLK, D], F32, tag="nat", name=f"qn{b}")
            kn = nat.tile([P, NBLK, D], F32, tag="nat", name=f"kn{b}")
            vn = nat.tile([P, NBLK, D], F32, tag="nat", name=f"vn{b}")
            nc.sync.dma_start(kn[:, 0:8, :], k_r[b][:, 0:8, :])
            nc.sync.dma_start(qn[:, 0:8, :], q_r[b][:, 0:8, :])
            nc.sync.dma_start(kn[:, 8:16, :], k_r[b][:, 8:16, :])
            nc.sync.dma_start(qn[:, 8:16, :], q_r[b][:, 8:16, :])
            nc.sync.dma_start(vn[:], v_r[b][:])
            nat_tiles[b] = (qn, kn, vn)

        def stage_casts(b):
            """bf16 casts on Pool (SBUF-only engine) for batch b."""
            qn, kn, vn = nat_tiles[b]
            kb = bfp.tile([P, NBLK, D], BF16, tag="bf", name=f"kb{b}")
            qb = bfp.tile([P, NBLK, D], BF16, tag="bf", name=f"qb{b}")
            nc.gpsimd.tensor_copy(kb[:], kn[:])
            nc.gpsimd.tensor_copy(qb[:], qn[:])
            vs = vp.tile([P, NBLK, D + 2], BF16, name=f"vs{b}")  # pad to 4B stride
            nc.gpsimd.tensor_copy(vs[:, :, 0:D], vn[:])
            nc.gpsimd.tensor_copy(vs[:, :, D:D + 1], ones_col[:])
            # zero-padded stationary tiles: HW rejects 64-row bf16 matmuls,
            # so S^T uses full-128-contraction operands with one half zeroed
            # (same PE cost: matmul time = out columns only).
            kte = ktp.tile([P, PAIRS, P], BF16, tag="kt", name=f"kte{b}")
            kto = ktp.tile([P, PAIRS, P], BF16, tag="kt", name=f"kto{b}")
            nc.gpsimd.memset(kte[D:P, :, :], 0.0)
            nc.gpsimd.memset(kto[0:D, :, :], 0.0)
            q_bf[b], v_sb[b] = qb, vs
            k_bf[b] = kb
            kt_all[b] = (kte, kto)

        def stage_transposes(b):
            """DMA xbar transposes (casts long done, so no SP queue blocking).

            Pair layout (the xbar's native J=128 form, verified on HW):
              kt[0:64, i, :] = k-chunk 2i ^T, kt[64:128, i, :] = chunk 2i+1 ^T
            qt_a same for q; qt_b = partition-swapped copy (odd chunk on
            0:64, even on 64:128) so every (k-parity, q-parity) matmul combo
            has base-aligned operands.
            """
            kb, qb = k_bf[b], q_bf[b]
            kte, kto = kt_all[b]
            ktp_pair = ktp.tile([P, PAIRS, P], BF16, tag="kt", name=f"ktp{b}")
            qa = qtp.tile([P, PAIRS, P], BF16, tag="qt", name=f"qa{b}")
            qb_t = qtp.tile([P, PAIRS, P], BF16, tag="qt", name=f"qb{b}")
            nc.sync.dma_start_transpose(ktp_pair[:], kb[:].rearrange("p c d -> p (c d)"))
            nc.sync.dma_start_transpose(qa[:], qb[:].rearrange("p c d -> p (c d)"))
            nc.gpsimd.tensor_copy(kte[0:D, :, :], ktp_pair[0:D, :, :])
            nc.gpsimd.tensor_copy(kto[D:P, :, :], ktp_pair[D:P, :, :])
            nc.vector.tensor_copy(qb_t[0:D, :, :], qa[D:P, :, :])
            nc.vector.tensor_copy(qb_t[D:P, :, :], qa[0:D, :, :])
            qt_all[b] = (qa, qb_t)

        units = [(b, qt) for b in range(B) for qt in range(NQT)]
        NU = len(units)
        st_tiles = {}

        def emit_st(g):
            """S^T for one group of 4 chunks (2 row-packed pairs).

            All matmuls are full-128-contraction, base partition 0: the
            unused half of each stationary kt tile is zeros, so the matching
            rhs half (the other q parity) contributes nothing.
            st[:, t, 0:128] = chunk (4j+t) x q-chunk 2e; 128:256 x 2e+1.
            """
            u, j = divmod(g, GROUPS)
            b, qt = units[u]
            e = qt  # q pair index
            qa, qb_t = qt_all[b]
            kte, kto = kt_all[b]
            st = ps_st.tile([P, 2, LQT], F32, tag="st", name=f"st{g}")
            for t in range(1):
                i = j  # k pair index (1 pair per dispatch group)
                te, to = 0, 1  # st slots for chunks 2i, 2i+1
                nc.tensor.matmul(
                    st[:, te, 0:P], kte[:, i, :], qa[:, e, :],
                    start=True, stop=True,
                )
                nc.tensor.matmul(
                    st[:, te, P:LQT], kte[:, i, :], qb_t[:, e, :],
                    start=True, stop=True,
                )
                nc.tensor.matmul(
                    st[:, to, 0:P], kto[:, i, :], qb_t[:, e, :],
                    start=True, stop=True,
                )
                nc.tensor.matmul(
                    st[:, to, P:LQT], kto[:, i, :], qa[:, e, :],
                    start=True, stop=True,
                )
            st_tiles[g] = st

        # ---- batch 0: halved staging, DVE casts for half 0 (Pool for the
        # rest) so the first S^T groups start ~5us in instead of ~18us.
        qn = nat.tile([P, NBLK, D], F32, tag="nat", name="qn0")
        kn = nat.tile([P, NBLK, D], F32, tag="nat", name="kn0")
        vn = nat.tile([P, NBLK, D], F32, tag="nat", name="vn0")
        nat_tiles[0] = (qn, kn, vn)
        qb0 = bfp.tile([P, NBLK, D], BF16, tag="bf", name="qb0")
        kte0 = ktp.tile([P, PAIRS, P], BF16, tag="kt", name="kte0")
        kto0 = ktp.tile([P, PAIRS, P], BF16, tag="kt", name="kto0")
        nc.vector.memset(kte0[D:P, :, :], 0.0)
        nc.vector.memset(kto0[0:D, :, :], 0.0)
        qa0 = qtp.tile([P, PAIRS, P], BF16, tag="qt", name="qa0")
        qbt0 = qtp.tile([P, PAIRS, P], BF16, tag="qt", name="qbt0")
        vs0 = vp.tile([P, NBLK, D + 2], BF16, name="vs0")  # pad to 4B stride
        # Batch 0 startup: PE-transpose (f32, PE is idle) the chunks the first
        # units need, straight off the loads; only q pairs 4:8 go through the
        # steady-state Pool-cast + DMA-xbar path.
        identf = const.tile([P, P], F32)
        make_identity(nc, identf[:])
        nc.sync.dma_start(qn[:, 0:2, :], q_r[0][:, 0:2, :])
        nc.sync.dma_start(kn[:, 0:8, :], k_r[0][:, 0:8, :])
        nc.sync.dma_start(vn[:, 0:4, :], v_r[0][:, 0:4, :])
        nc.sync.dma_start(kn[:, 8:16, :], k_r[0][:, 8:16, :])
        nc.sync.dma_start(qn[:, 2:16, :], q_r[0][:, 2:16, :])
        nc.sync.dma_start(vn[:, 4:16, :], v_r[0][:, 4:16, :])

        def pe_pair_transpose(tp, i, src):
            nc.tensor.transpose(
                tp, src[:, 2 * i: 2 * i + 2, :].rearrange("p c d -> p (c d)"),
                identf[:],
            )

        # q pair 0 first (gates the very first S^T)
        tpq0 = ps_st.tile([P, P], F32, tag="st", name="tpq0")
        pe_pair_transpose(tpq0[:], 0, qn)
        nc.vector.tensor_copy(qa0[:, 0, :], tpq0[:])
        nc.vector.tensor_copy(qbt0[0:D, 0, :], tpq0[D:P, :])
        nc.vector.tensor_copy(qbt0[D:P, 0, :], tpq0[0:D, :])
        nc.vector.tensor_copy(vs0[:, 0:4, 0:D], vn[:, 0:4, :])
        nc.vector.tensor_copy(vs0[:, 0:4, D:D + 1], ones_col[:, 0:4, :])
        # k pairs 0:4 then 4:8 (Act does the PSUM->SBUF cast copies)
        for h in range(2):
            tpk = ps_st.tile([P, 4, P], F32, tag="st", name=f"tpk{h}")
            for i in range(4):
                pe_pair_transpose(tpk[:, i, :], 4 * h + i, kn)
            nc.scalar.activation(
                kte0[0:D, 4 * h: 4 * h + 4, :], tpk[0:D, :, :],
                mybir.ActivationFunctionType.Copy,
            )
            nc.scalar.activation(
                kto0[D:P, 4 * h: 4 * h + 4, :], tpk[D:P, :, :],
                mybir.ActivationFunctionType.Copy,
            )
        # q pairs 1:4
        tpq1 = ps_st.tile([P, 3, P], F32, tag="st", name="tpq1")
        for i in range(1, 4):
            pe_pair_transpose(tpq1[:, i - 1, :], i, qn)
        nc.vector.tensor_copy(qa0[:, 1:4, :], tpq1[:])
        nc.vector.tensor_copy(qbt0[0:D, 1:4, :], tpq1[D:P, :, :])
        nc.vector.tensor_copy(qbt0[D:P, 1:4, :], tpq1[0:D, :, :])
        # q pairs 4:8 via the steady-state path
        if USE_DMAT:
            nc.gpsimd.tensor_copy(qb0[:, 8:16, :], qn[:, 8:16, :])
            nc.sync.dma_start_transpose(
                qa0[:, 4:8, :], qb0[:, 8:16, :].rearrange("p c d -> p (c d)")
            )
        else:
            tpq2 = ps_st.tile([P, 4, P], F32, tag="st", name="tpq0b")
            for i in range(4, 8):
                pe_pair_transpose(tpq2[:, i - 4, :], i, qn)
            nc.vector.tensor_copy(qa0[:, 4:8, :], tpq2[:])
        nc.vector.tensor_copy(qbt0[0:D, 4:8, :], qa0[D:P, 4:8, :])
        nc.vector.tensor_copy(qbt0[D:P, 4:8, :], qa0[0:D, 4:8, :])
        nc.gpsimd.tensor_copy(vs0[:, 4:16, 0:D], vn[:, 4:16, :])
        nc.gpsimd.tensor_copy(vs0[:, 4:16, D:D + 1], ones_col[:, 4:16, :])
        q_bf[0], kt_all[0], v_sb[0] = qb0, (kte0, kto0), vs0
        qt_all[0] = (qa0, qbt0)

        emit_loads(1)
        for g in range(ST_AHEAD):
            emit_st(g)

        COPY = mybir.ActivationFunctionType.Copy
        pending_store = [None]  # delayed one group so evac doesn't block Act

        def flush_store():
            if pending_store[0] is None:
                return
            oT_p, b_p, qt_p, u_p = pending_store[0]
            oT_sb = otp.tile([D + 1, LQT], F32, tag="ot_sb", name=f"os{u_p}")
            nc.scalar.activation(oT_sb[:], oT_p[:], COPY)
            nc.sync.dma_start(
                o_d.ap()[b_p, :, qt_p * LQT:(qt_p + 1) * LQT], oT_sb[:]
            )
            pending_store[0] = None

        for u, (b, qt) in enumerate(units):
            if qt == 0 and b + 1 < B:
                stage_casts(b + 1)
            if qt == 3 and b + 1 < B:
                stage_transposes(b + 1)
            if qt == 1 and b + 2 < B:
                emit_loads(b + 2)

            oT = ps_ot.tile([D + 1, LQT], F32, tag="ot", name=f"ot{u}")
            for j in range(GROUPS):
                g = u * GROUPS + j
                st = st_tiles.pop(g)
                if ENG_MAP[j] == "A":
                    pg = pp.tile([P, 2, LQT], BF16, tag="pg", name=f"pg{g}")
                    nc.scalar.activation(pg[:], st[:], EXP, scale=SCALE)
                    rhs = [pg[:, t, :] for t in range(2)]
                else:
                    pg = pp.tile([P, 2, LQT], I16, tag="pg", name=f"pg{g}")
                    nc.vector.tensor_scalar(pg[:], st[:], A16, B16, MULT, ADD)
                    rhs = [pg[:, t, :].bitcast(BF16) for t in range(2)]
                for t in range(2):
                    c = 2 * j + t  # chunk index
                    nc.tensor.matmul(
                        oT[:], v_sb[b][:, c, 0:D + 1], rhs[t],
                        start=(c == 0), stop=(c == NBLK - 1),
                    )
                if g + ST_AHEAD < NU * GROUPS:
                    emit_st(g + ST_AHEAD)
                if j == 0:
                    flush_store()

            pending_store[0] = (oT, b, qt, u)

        flush_store()

    nc.compile()
    return nc


_NC_CACHE = None


def _get_nc():
    global _NC_CACHE
    if _NC_CACHE is None:
        _NC_CACHE = build_attention_kernel()
    return _NC_CACHE


def kernel(q, k, v):
    from concourse import bass_utils

    q = np.ascontiguousarray(np.asarray(q, dtype=np.float32))
    k = np.ascontiguousarray(np.asarray(k, dtype=np.float32))
    v = np.ascontiguousarray(np.asarray(v, dtype=np.float32))
    assert q.shape == (B_TOTAL, L, D), q.shape

    nc = _get_nc()
    in_maps = [
        {
            "q": q[i * B: (i + 1) * B],
            "k": k[i * B: (i + 1) * B],
            "v": v[i * B: (i + 1) * B],
        }
        for i in range(N_CORES)
    ]
    res = bass_utils.run_bass_kernel_spmd(nc, in_maps, core_ids=list(range(N_CORES)))
    outt = np.concatenate(
        [res.results[i]["outt"] for i in range(N_CORES)], axis=0
    )
    out = outt[:, :D, :] / outt[:, D:D + 1, :]
    return np.ascontiguousarray(out.transpose(0, 2, 1))


# revision 16
# speedup vs baseline: 1.0257x; 1.0257x over previous
"""Batched dot-product attention (B=32, Lq=Lk=2048, d=dv=64, fp32) on 8 TRN2
NeuronCores.

Data parallel over batch (4 per core). Transposed-layout attention, bf16
matmul operands / f32 PSUM accumulation:

  - q, k, v are cast to bf16 on the Pool engine (GPSIMD cannot touch PSUM on
    real silicon, so it gets all SBUF-side prep).
  - q^T / k^T are built by the DMA xbar transpose engine (dma_start_transpose,
    2-byte dtypes) straight from the natural [p, chunk, d] bf16 staging into
    the pair layout out[j, c, p] = in[p, c*128 + j] (J=128 is the only form
    whose HW semantics match the AP). The PE runs matmuls only (plus a few
    f32 transposes to jump-start batch 0 while the DMA path warms up).
  - S^T chunk [128 (Lk), 256 (Lq)] matmuls use full-128-contraction operands
    with one half zeroed (kte = even k-chunks on rows 0:64 + zeros, kto =
    odd chunks on rows 64:128 + zeros; qa / partition-swapped qb_t supply the
    matching q parity). Real HW rejects 64-row bf16 matmuls, and zero-padding
    is free: matmul cost is out-columns only.
  - exp (16.8M elems/core, the scalar-class bottleneck) is split per
    chunk-pair, ENG_MAP = Act x5 / DVE x3 per q tile:
      Act: true exp activation, bf16 out, scale=1/sqrt(d) folded
      DVE: Schraudolph bit trick, one op: int16(x*A16 + B16) bitcast to bf16
           (mult err < ~3.3%; softmax normalization cancels most of it)
  - out^T [65, 256] accumulates over the 16 Lk chunks in f32 PSUM with
    lhsT = [v_chunk | ones]; row 64 is the softmax denominator for free.
    Final divide + transpose back to [L, 64] happen on host (trivial numpy).
  - The S^T producer runs 6 pairs ahead of the exp/PV consumer (6 one-bank
    S^T tiles + 2 out^T banks = all 8 PSUM banks) so the PE (~88% busy, the
    bottleneck engine) rarely waits; out^T evacuation (Act) is delayed one
    dispatch group so it never blocks the next unit's exp.

k and v use a row permutation (DRAM rearrange "(p c) d") for 2-4KB DMA runs;
the permutation is consistent between k-chunks and v-chunks so the softmax
math is unchanged. q keeps natural order (its order defines output rows).
No max-subtraction: raw logits are within +-70, safely inside both the f32
exp range and the bit-trick's int16 domain.
"""

import sys

if "/opt/trn_rl_repo" not in sys.path:
    sys.path.insert(0, "/opt/trn_rl_repo")

from contextlib import ExitStack

import numpy as np

import concourse.tile as tile
from concourse import bacc, mybir
from concourse.masks import make_identity

B_TOTAL = 32
N_CORES = 8
B = B_TOTAL // N_CORES
L = 2048
D = 64
P = 128
NBLK = L // P  # 16 Lk chunks
LQT = 256
NQT = L // LQT  # 8 q tiles per batch
PAIRS = NBLK // 2  # 8 chunk pairs per q tile
SCALE = 1.0 / float(np.sqrt(D))

F32 = mybir.dt.float32
BF16 = mybir.dt.bfloat16
I16 = mybir.dt.int16
EXP = mybir.ActivationFunctionType.Exp
MULT = mybir.AluOpType.mult
ADD = mybir.AluOpType.add

# Schraudolph bf16 bit trick: int16 bits = trunc(x*A16 + B16); bits viewed as
# bf16 ~= exp(x/8) within ~3.3%.
A16 = float(SCALE * np.log2(np.e) * (2 ** 7))
B16 = float((127.0 - 0.044) * (2 ** 7))

ENG_MAP = "AADADADA"  # pair -> engine: Act x5, DVE x3 per q tile
USE_DMAT = True  # DMA xbar transposes (False: PE transposes, for bisecting)
GROUPS = 8  # dispatch groups (1 chunk-pair each) per q tile
ST_AHEAD = 6  # S^T producer lookahead in groups == ps_st bufs


def build_attention_kernel():
    nc = bacc.Bacc("TRN2", target_bir_lowering=False, debug=False)
    q_d = nc.dram_tensor("q", [B, L, D], F32, kind="ExternalInput")
    k_d = nc.dram_tensor("k", [B, L, D], F32, kind="ExternalInput")
    v_d = nc.dram_tensor("v", [B, L, D], F32, kind="ExternalInput")
    o_d = nc.dram_tensor("outt", [B, D + 1, L], F32, kind="ExternalOutput")

    q_r = [q_d.ap()[b].rearrange("(c p) d -> p c d", p=P) for b in range(B)]
    k_r = [k_d.ap()[b].rearrange("(p c) d -> p c d", p=P) for b in range(B)]
    v_r = [v_d.ap()[b].rearrange("(p c) d -> p c d", p=P) for b in range(B)]

    with tile.TileContext(nc) as tc, ExitStack() as ctx:
        const = ctx.enter_context(tc.tile_pool(name="const", bufs=1))
        nat = ctx.enter_context(tc.tile_pool(name="nat", bufs=6))
        bfp = ctx.enter_context(tc.tile_pool(name="bfp", bufs=4))
        ktp = ctx.enter_context(tc.tile_pool(name="ktp", bufs=6))
        qtp = ctx.enter_context(tc.tile_pool(name="qtp", bufs=4))
        vp = ctx.enter_context(tc.tile_pool(name="vp", bufs=2))
        pp = ctx.enter_context(tc.tile_pool(name="pp", bufs=8))
        otp = ctx.enter_context(tc.tile_pool(name="otp", bufs=3))
        ps_st = ctx.enter_context(tc.tile_pool(name="ps_st", bufs=ST_AHEAD, space="PSUM"))
        ps_ot = ctx.enter_context(tc.tile_pool(name="ps_ot", bufs=2, space="PSUM"))

        ident = const.tile([P, P], BF16)
        make_identity(nc, ident[:])
        ones_col = const.tile([P, NBLK, 1], BF16)
        nc.vector.memset(ones_col[:], 1.0)

        # Warm the PE p-state during initial DMA/staging.
        warm = ps_st.tile([P, 2, P], BF16, tag="st", name="warm")
        for w in range(8):
            nc.tensor.transpose(warm[:, w % 2, :], ident[:], ident[:])

        q_bf = [None] * B
        k_bf = [None] * B
        kt_all = [None] * B
        qt_all = [None] * B
        v_sb = [None] * B
        nat_tiles = [None] * B

        def emit_loads(b):
            qn = nat.tile([P, NBLK, D], F32, tag="nat", name=f"qn{b}")
            kn = nat.tile([P, NBLK, D], F32, tag="nat", name=f"kn{b}")
            vn = nat.tile([P, NBLK, D], F32, tag="nat", name=f"vn{b}")
            nc.sync.dma_start(kn[:, 0:8, :], k_r[b][:, 0:8, :])
            nc.sync.dma_start(qn[:, 0:8, :], q_r[b][:, 0:8, :])
            nc.sync.dma_start(kn[:, 8:16, :], k_r[b][:, 8:16, :])
            nc.sync.dma_start(qn[:, 8:16, :], q_r[b][:, 8:16, :])
            nc.sync.dma_start(vn[:], v_r[b][:])
            nat_tiles[b] = (qn, kn, vn)

        def stage_casts(b):
            """bf16 casts on Pool (SBUF-only engine) for batch b."""
            qn, kn, vn = nat_tiles[b]
            kb = bfp.tile([P, NBLK, D], BF16, tag="bf", name=f"kb{b}")
            qb = bfp.tile([P, NBLK, D], BF16, tag="bf", name=f"qb{b}")
            nc.gpsimd.tensor_copy(kb[:], kn[:])
            nc.gpsimd.tensor_copy(qb[:], qn[:])
            vs = vp.tile([P, NBLK, D + 2], BF16, name=f"vs{b}")  # pad to 4B stride
            nc.gpsimd.tensor_copy(vs[:, :, 0:D], vn[:])
            nc.gpsimd.tensor_copy(vs[:, :, D:D + 1], ones_col[:])
            # zero-padded stationary tiles: HW rejects 64-row bf16 matmuls,
            # so S^T uses full-128-contraction operands with one half zeroed
            # (same PE cost: matmul time = out columns only).
            kte = ktp.tile([P, PAIRS, P], BF16, tag="kt", name=f"kte{b}")
            kto = ktp.tile([P, PAIRS, P], BF16, tag="kt", name=f"kto{b}")
            nc.gpsimd.memset(kte[D:P, :, :], 0.0)
            nc.gpsimd.memset(kto[0:D, :, :], 0.0)
            q_bf[b], v_sb[b] = qb, vs
            k_bf[b] = kb
            kt_all[b] = (kte, kto)

        def stage_transposes(b):
            """DMA xbar transposes (casts long done, so no SP queue blocking).

            Pair layout (the xbar's native J=128 form, verified on HW):
              kt[0:64, i, :] = k-chunk 2i ^T, kt[64:128, i, :] = chunk 2i+1 ^T
            qt_a same for q; qt_b = partition-swapped copy (odd chunk on
            0:64, even on 64:128) so every (k-parity, q-parity) matmul combo
            has base-aligned operands.
            """
            kb, qb = k_bf[b], q_bf[b]
            kte, kto = kt_all[b]
            ktp_pair = ktp.tile([P, PAIRS, P], BF16, tag="kt", name=f"ktp{b}")
            qa = qtp.tile([P, PAIRS, P], BF16, tag="qt", name=f"qa{b}")
            qb_t = qtp.tile([P, PAIRS, P], BF16, tag="qt", name=f"qb{b}")
            nc.sync.dma_start_transpose(ktp_pair[:], kb[:].rearrange("p c d -> p (c d)"))
            nc.sync.dma_start_transpose(qa[:], qb[:].rearrange("p c d -> p (c d)"))
            nc.gpsimd.tensor_copy(kte[0:D, :, :], ktp_pair[0:D, :, :])
            nc.gpsimd.tensor_copy(kto[D:P, :, :], ktp_pair[D:P, :, :])
            nc.vector.tensor_copy(qb_t[0:D, :, :], qa[D:P, :, :])
            nc.vector.tensor_copy(qb_t[D:P, :, :], qa[0:D, :, :])
            qt_all[b] = (qa, qb_t)

        units = [(b, qt) for b in range(B) for qt in range(NQT)]
        NU = len(units)
        st_tiles = {}

        def emit_st(g):
            """S^T for one group of 4 chunks (2 row-packed pairs).

            All matmuls are full-128-contraction, base partition 0: the
            unused half of each stationary kt tile is zeros, so the matching
            rhs half (the other q parity) contributes nothing.
            st[:, t, 0:128] = chunk (4j+t) x q-chunk 2e; 128:256 x 2e+1.
            """
            u, j = divmod(g, GROUPS)
            b, qt = units[u]
            e = qt  # q pair index
            qa, qb_t = qt_all[b]
            kte, kto = kt_all[b]
            st = ps_st.tile([P, 2, LQT], F32, tag="st", name=f"st{g}")
            for t in range(1):
                i = j  # k pair index (1 pair per dispatch group)
                te, to = 0, 1  # st slots for chunks 2i, 2i+1
                nc.tensor.matmul(
                    st[:, te, 0:P], kte[:, i, :], qa[:, e, :],
                    start=True, stop=True,
                )
                nc.tensor.matmul(
                    st[:, te, P:LQT], kte[:, i, :], qb_t[:, e, :],
                    start=True, stop=True,
                )
                nc.tensor.matmul(
                    st[:, to, 0:P], kto[:, i, :], qb_t[:, e, :],
                    start=True, stop=True,
                )
                nc.tensor.matmul(
                    st[:, to, P:LQT], kto[:, i, :], qa[:, e, :],
                    start=True, stop=True,
                )
            st_tiles[g] = st

        # ---- batch 0: halved staging, DVE casts for half 0 (Pool for the
        # rest) so the first S^T groups start ~5us in instead of ~18us.
        qn = nat.tile([P, NBLK, D], F32, tag="nat", name="qn0")
        kn = nat.tile([P, NBLK, D], F32, tag="nat", name="kn0")
        vn = nat.tile([P, NBLK, D], F32, tag="nat", name="vn0")
        nat_tiles[0] = (qn, kn, vn)
        qb0 = bfp.tile([P, NBLK, D], BF16, tag="bf", name="qb0")
        kte0 = ktp.tile([P, PAIRS, P], BF16, tag="kt", name="kte0")
        kto0 = ktp.tile([P, PAIRS, P], BF16, tag="kt", name="kto0")
        nc.vector.memset(kte0[D:P, :, :], 0.0)
        nc.vector.memset(kto0[0:D, :, :], 0.0)
        qa0 = qtp.tile([P, PAIRS, P], BF16, tag="qt", name="qa0")
        qbt0 = qtp.tile([P, PAIRS, P], BF16, tag="qt", name="qbt0")
        vs0 = vp.tile([P, NBLK, D + 2], BF16, name="vs0")  # pad to 4B stride
        # Batch 0 startup: PE-transpose (f32, PE is idle) the chunks the first
        # units need, straight off the loads; only q pairs 4:8 go through the
        # steady-state Pool-cast + DMA-xbar path.
        identf = const.tile([P, P], F32)
        make_identity(nc, identf[:])
        nc.sync.dma_start(qn[:, 0:2, :], q_r[0][:, 0:2, :])
        nc.sync.dma_start(kn[:, 0:8, :], k_r[0][:, 0:8, :])
        nc.sync.dma_start(vn[:, 0:4, :], v_r[0][:, 0:4, :])
        nc.sync.dma_start(kn[:, 8:16, :], k_r[0][:, 8:16, :])
        nc.sync.dma_start(qn[:, 2:16, :], q_r[0][:, 2:16, :])
        nc.sync.dma_start(vn[:, 4:16, :], v_r[0][:, 4:16, :])

        def pe_pair_transpose(tp, i, src):
            nc.tensor.transpose(
                tp, src[:, 2 * i: 2 * i + 2, :].rearrange("p c d -> p (c d)"),
                identf[:],
            )

        # q pair 0 first (gates the very first S^T)
        tpq0 = ps_st.tile([P, P], F32, tag="st", name="tpq0")
        pe_pair_transpose(tpq0[:], 0, qn)
        nc.vector.tensor_copy(qa0[:, 0, :], tpq0[:])
        nc.vector.tensor_copy(qbt0[0:D, 0, :], tpq0[D:P, :])
        nc.vector.tensor_copy(qbt0[D:P, 0, :], tpq0[0:D, :])
        nc.vector.tensor_copy(vs0[:, 0:4, 0:D], vn[:, 0:4, :])
        nc.vector.tensor_copy(vs0[:, 0:4, D:D + 1], ones_col[:, 0:4, :])
        # k pairs 0:4 then 4:8 (Act does the PSUM->SBUF cast copies)
        for h in range(2):
            tpk = ps_st.tile([P, 4, P], F32, tag="st", name=f"tpk{h}")
            for i in range(4):
                pe_pair_transpose(tpk[:, i, :], 4 * h + i, kn)
            nc.scalar.activation(
                kte0[0:D, 4 * h: 4 * h + 4, :], tpk[0:D, :, :],
                mybir.ActivationFunctionType.Copy,
            )
            nc.scalar.activation(
                kto0[D:P, 4 * h: 4 * h + 4, :], tpk[D:P, :, :],
                mybir.ActivationFunctionType.Copy,
            )
        # q pairs 1:4
        tpq1 = ps_st.tile([P, 3, P], F32, tag="st", name="tpq1")
        for i in range(1, 4):
            pe_pair_transpose(tpq1[:, i - 1, :], i, qn)
        nc.vector.tensor_copy(qa0[:, 1:4, :], tpq1[:])
        nc.vector.tensor_copy(qbt0[0:D, 1:4, :], tpq1[D:P, :, :])
        nc.vector.tensor_copy(qbt0[D:P, 1:4, :], tpq1[0:D, :, :])
        # q pairs 4:8 via the steady-state path
        if USE_DMAT:
            nc.gpsimd.tensor_copy(qb0[:, 8:16, :], qn[:, 8:16, :])
            nc.sync.dma_start_transpose(
                qa0[:, 4:8, :], qb0[:, 8:16, :].rearrange("p c d -> p (c d)")
            )
        else:
            tpq2 = ps_st.tile([P, 4, P], F32, tag="st", name="tpq0b")
            for i in range(4, 8):
                pe_pair_transpose(tpq2[:, i - 4, :], i, qn)
            nc.vector.tensor_copy(qa0[:, 4:8, :], tpq2[:])
        nc.vector.tensor_copy(qbt0[0:D, 4:8, :], qa0[D:P, 4:8, :])
        nc.vector.tensor_copy(qbt0[D:P, 4:8, :], qa0[0:D, 4:8, :])
        nc.gpsimd.tensor_copy(vs0[:, 4:16, 0:D], vn[:, 4:16, :])
        nc.gpsimd.tensor_copy(vs0[:, 4:16, D:D + 1], ones_col[:, 4:16, :])
        q_bf[0], kt_all[0], v_sb[0] = qb0, (kte0, kto0), vs0
        qt_all[0] = (qa0, qbt0)

        emit_loads(1)
        for g in range(ST_AHEAD):
            emit_st(g)

        COPY = mybir.ActivationFunctionType.Copy
        pending_store = [None]  # delayed one group so evac doesn't block Act

        def flush_store():
            if pending_store[0] is None:
                return
            oT_p, b_p, qt_p, u_p = pending_store[0]
            oT_sb = otp.tile([D + 1, LQT], F32, tag="ot_sb", name=f"os{u_p}")
            nc.scalar.activation(oT_sb[:], oT_p[:], COPY)
            nc.sync.dma_start(
                o_d.ap()[b_p, :, qt_p * LQT:(qt_p + 1) * LQT], oT_sb[:]
            )
            pending_store[0] = None

        for u, (b, qt) in enumerate(units):
            if qt == 0 and b + 1 < B:
                stage_casts(b + 1)
            if qt == 3 and b + 1 < B:
                stage_transposes(b + 1)
            if qt == 1 and b + 2 < B:
                emit_loads(b + 2)

            oT = ps_ot.tile([D + 1, LQT], F32, tag="ot", name=f"ot{u}")
            for j in range(GROUPS):
                g = u * GROUPS + j
                st = st_tiles.pop(g)
                if ENG_MAP[j] == "A":
                    pg = pp.tile([P, 2, LQT], BF16, tag="pg", name=f"pg{g}")
                    nc.scalar.activation(pg[:], st[:], EXP, scale=SCALE)
                    rhs = [pg[:, t, :] for t in range(2)]
                else:
                    pg = pp.tile([P, 2, LQT], I16, tag="pg", name=f"pg{g}")
                    nc.vector.tensor_scalar(pg[:], st[:], A16, B16, MULT, ADD)
                    rhs = [pg[:, t, :].bitcast(BF16) for t in range(2)]
                for t in range(2):
                    c = 2 * j + t  # chunk index
                    nc.tensor.matmul(
                        oT[:], v_sb[b][:, c, 0:D + 1], rhs[t],
                        start=(c == 0), stop=(c == NBLK - 1),
                    )
                if g + ST_AHEAD < NU * GROUPS:
                    emit_st(g + ST_AHEAD)
                if j == 0:
                    flush_store()

            pending_store[0] = (oT, b, qt, u)

        flush_store()

    nc.compile()
    return nc


_NC_CACHE = None


def _get_nc():
    global _NC_CACHE
    if _NC_CACHE is None:
        _NC_CACHE = build_attention_kernel()
    return _NC_CACHE


def kernel(q, k, v):
    from concourse import bass_utils

    q = np.ascontiguousarray(np.asarray(q, dtype=np.float32))
    k = np.ascontiguousarray(np.asarray(k, dtype=np.float32))
    v = np.ascontiguousarray(np.asarray(v, dtype=np.float32))
    assert q.shape == (B_TOTAL, L, D), q.shape

    nc = _get_nc()
    in_maps = [
        {
            "q": q[i * B: (i + 1) * B],
            "k": k[i * B: (i + 1) * B],
            "v": v[i * B: (i + 1) * B],
        }
        for i in range(N_CORES)
    ]
    res = bass_utils.run_bass_kernel_spmd(nc, in_maps, core_ids=list(range(N_CORES)))
    outt = np.concatenate(
        [res.results[i]["outt"] for i in range(N_CORES)], axis=0
    )
    out = outt[:, :D, :] / outt[:, D:D + 1, :]
    return np.ascontiguousarray(out.transpose(0, 2, 1))
